# revision 4
# baseline (speedup 1.0000x reference)
"""TRN2 Bass kernel for nn_Actor (retrieval_knn).

Data-parallel over batch across 8 NeuronCores (8192 rows/core).
Per core: ap_gather embedding lookup (feature-major), fp32 MLP layer-1 on
TensorE, bf16-split exact scores vs the 2489-row table with W2 absorbed
(scores = h @ (table@W2).T + table@b2), and a packed-fp32 argmax:
the PE quantizes scores to a 2^-5 grid via a 2^18 bias row and adds the
candidate index in the low mantissa bits (n * 2^-17); a single DVE max8
scan per tile yields top-8 packed values; the host decodes indices and
rescores the <=8 candidates per row exactly to break quantization ties.
"""
import sys
sys.path.insert(0, '/opt/trn_rl_repo')
import numpy as np
import ml_dtypes

B = 65536
NCORES = 8
BC = B // NCORES            # 8192
NW, NPTAB, EMB = 1807, 2490, 10
NPROJ = NPTAB - 1           # 2489
HID = 40
BIGQ = np.float32(2.0 ** 18)
EPS = 2.0 ** -17
UNITS = [(0, 1024), (1024, 1024), (2048, NPROJ - 2048)]   # psum units per 128-row tile
NTILES = BC // 128          # 64

_cache = {}


def _bf16(x):
    return np.asarray(x, np.float32).astype(ml_dtypes.bfloat16)


def _build(L=1):
    from concourse import bacc, mybir
    from concourse.tile import TileContext
    dt = mybir.dt
    nc = bacc.Bacc("TRN2", target_bir_lowering=False, debug=False, num_devices=NCORES)

    widx = nc.dram_tensor("widx", [128, BC // 8 // 16], dt.int16, kind="ExternalInput")
    pidx = nc.dram_tensor("pidx", [128, BC // 8 // 16], dt.int16, kind="ExternalInput")
    wtab = nc.dram_tensor("wtab", [128, NW], dt.float32, kind="ExternalInput")
    ptab = nc.dram_tensor("ptab", [128, NPTAB], dt.float32, kind="ExternalInput")
    w1t = nc.dram_tensor("w1t", [20, HID], dt.float32, kind="ExternalInput")
    b1e = nc.dram_tensor("b1e", [HID, 1], dt.float32, kind="ExternalInput")
    tstk = nc.dram_tensor("tstk", [123, NPROJ], dt.bfloat16, kind="ExternalInput")
    b1r = nc.dram_tensor("b1r", [2, NPROJ], dt.bfloat16, kind="ExternalInput")
    b2r = nc.dram_tensor("b2r", [2, NPROJ], dt.bfloat16, kind="ExternalInput")
    ones3 = nc.dram_tensor("ones3", [3, BC], dt.bfloat16, kind="ExternalInput")
    out_ext = nc.dram_tensor("out", [128, NTILES * 8], dt.float32, kind="ExternalOutput")

    NIDX = BC // 8           # ids per gather group = 1024
    with TileContext(nc) as tc:
        with tc.tile_pool(name="const", bufs=1) as cp, \
             tc.tile_pool(name="work", bufs=1) as wp, \
             tc.tile_pool(name="hr", bufs=2) as hrp, \
             tc.tile_pool(name="m8", bufs=4) as m8p, \
             tc.tile_pool(name="hm", bufs=2, space="PSUM") as hmp, \
             tc.tile_pool(name="sc", bufs=3, space="PSUM") as scp:
            t_wtab = cp.tile([128, NW], dt.float32)
            t_ptab = cp.tile([128, NPTAB], dt.float32)
            t_widx = cp.tile([128, NIDX // 16], dt.int16)
            t_pidx = cp.tile([128, NIDX // 16], dt.int16)
            t_w1t = cp.tile([20, HID], dt.float32)
            t_b1 = cp.tile([HID, 1], dt.float32)
            t_tstk = cp.tile([123, NPROJ], dt.bfloat16)
            t_b1r = cp.tile([2, NPROJ], dt.bfloat16)
            t_b2r = cp.tile([2, NPROJ], dt.bfloat16)
            ones2 = cp.tile([2, 128], dt.bfloat16)
            nc.sync.dma_start(out=t_wtab, in_=wtab.ap())
            nc.sync.dma_start(out=t_ptab, in_=ptab.ap())
            nc.sync.dma_start(out=t_widx, in_=widx.ap())
            nc.sync.dma_start(out=t_pidx, in_=pidx.ap())
            nc.sync.dma_start(out=t_w1t, in_=w1t.ap())
            nc.sync.dma_start(out=t_b1, in_=b1e.ap())
            nc.sync.dma_start(out=t_tstk, in_=tstk.ap())
            nc.sync.dma_start(out=t_b1r, in_=b1r.ap())
            nc.sync.dma_start(out=t_b2r, in_=b2r.ap())
            nc.vector.memset(ones2, 1.0)

            wg = wp.tile([128, NIDX], dt.float32)
            pg = wp.tile([128, NIDX], dt.float32)
            x = wp.tile([20, BC], dt.float32)
            hstack = wp.tile([123, BC], dt.bfloat16)
            h_f32 = wp.tile([HID, BC], dt.float32)
            h1f = wp.tile([HID, BC], dt.float32)
            outbuf = wp.tile([128, NTILES * 8], dt.float32)
            nc.sync.dma_start(out=hstack[120:123, :], in_=ones3.ap())

            for _ in range(L):
                nc.gpsimd.ap_gather(out_ap=wg, in_ap=t_wtab, idxs_ap=t_widx,
                                    channels=128, num_elems=NW, d=1, num_idxs=NIDX)
                nc.gpsimd.ap_gather(out_ap=pg, in_ap=t_ptab, idxs_ap=t_pidx,
                                    channels=128, num_elems=NPTAB, d=1, num_idxs=NIDX)
                for g in range(8):
                    nc.gpsimd.dma_start(out=x[0:10, g * NIDX:(g + 1) * NIDX],
                                        in_=wg[16 * g:16 * g + 10, :])
                    nc.gpsimd.dma_start(out=x[10:20, g * NIDX:(g + 1) * NIDX],
                                        in_=pg[16 * g:16 * g + 10, :])

                for g in range(8):
                    g0 = g * NIDX
                    for cc in range(NIDX // 512):
                        c0 = g0 + cc * 512
                        hm = hmp.tile([HID, 512], dt.float32)
                        nc.tensor.matmul(hm, lhsT=t_w1t, rhs=x[:, c0:c0 + 512],
                                         start=True, stop=True)
                        import concourse.mybir as mb
                        nc.scalar.activation(hstack[0:HID, c0:c0 + 512], hm,
                                             mb.ActivationFunctionType.Relu, bias=t_b1)
                        nc.scalar.activation(h_f32[:, c0:c0 + 512], hm,
                                             mb.ActivationFunctionType.Relu, bias=t_b1)
                        nc.gpsimd.tensor_copy(h1f[:, c0:c0 + 512], hstack[0:HID, c0:c0 + 512])
                        hr = hrp.tile([HID, 512], dt.float32)
                        nc.gpsimd.tensor_sub(hr, h_f32[:, c0:c0 + 512], h1f[:, c0:c0 + 512])
                        nc.gpsimd.tensor_copy(hstack[64:104, c0:c0 + 512], hr)
                    nc.gpsimd.dma_start(out=hstack[40:64, g0:g0 + NIDX],
                                        in_=hstack[0:24, g0:g0 + NIDX])
                    nc.gpsimd.dma_start(out=hstack[104:120, g0:g0 + NIDX],
                                        in_=hstack[24:40, g0:g0 + NIDX])

                    for t in range(8 * g, 8 * (g + 1)):
                        lcols = slice(t * 128, (t + 1) * 128)
                        m8 = m8p.tile([128, 8 * len(UNITS)], dt.float32)
                        for u, (c0, cw) in enumerate(UNITS):
                            ps = scp.tile([128, 1024], dt.float32)
                            splits = [(s, min(512, cw - s)) for s in range(0, cw, 512)]
                            for s0, sw in splits:
                                nc.tensor.matmul(ps[:, s0:s0 + sw], lhsT=hstack[:, lcols],
                                                 rhs=t_tstk[:, c0 + s0:c0 + s0 + sw],
                                                 start=True, stop=False)
                            for s0, sw in splits:
                                nc.tensor.matmul(ps[:, s0:s0 + sw], lhsT=ones2,
                                                 rhs=t_b1r[:, c0 + s0:c0 + s0 + sw],
                                                 start=False, stop=False)
                            for s0, sw in splits:
                                nc.tensor.matmul(ps[:, s0:s0 + sw], lhsT=ones2,
                                                 rhs=t_b2r[:, c0 + s0:c0 + s0 + sw],
                                                 start=False, stop=True)
                            nc.vector.max(out=m8[:, 8 * u:8 * u + 8], in_=ps[:, 0:cw])
                        nc.vector.max(out=outbuf[:, 8 * t:8 * t + 8], in_=m8)

            nc.sync.dma_start(out=out_ext.ap(), in_=outbuf)
    nc.compile()
    return nc


def _host_prep(inputs):
    worker_ids = np.asarray(inputs["worker_ids"]).astype(np.int64)
    project_ids = np.asarray(inputs["project_ids"]).astype(np.int64)
    worker_emb = np.asarray(inputs["worker_emb"], dtype=np.float32)
    project_emb = np.asarray(inputs["project_emb"], dtype=np.float32)
    W1 = np.asarray(inputs["W1"], dtype=np.float32)
    b1 = np.asarray(inputs["b1"], dtype=np.float32)
    W2 = np.asarray(inputs["W2"], dtype=np.float32)
    b2 = np.asarray(inputs["b2"], dtype=np.float32)

    table = project_emb[1:]
    G = (table @ W2).astype(np.float32)
    c = (table @ b2).astype(np.float32)
    G1 = _bf16(G)
    G2 = _bf16(G - G1.astype(np.float32))
    c1 = _bf16(c)
    c2 = _bf16(c - c1.astype(np.float32))
    tstk = np.zeros((123, NPROJ), dtype=ml_dtypes.bfloat16)
    tstk[0:40] = G1.T
    tstk[40:64] = G2.T[0:24]
    tstk[64:104] = G1.T
    tstk[104:120] = G2.T[24:40]
    tstk[120] = c1
    tstk[121] = c2
    tstk[122] = ml_dtypes.bfloat16(BIGQ)
    b1r = np.zeros((2, NPROJ), dtype=ml_dtypes.bfloat16)
    b1r[0] = ml_dtypes.bfloat16(-BIGQ)
    b1r[1] = ml_dtypes.bfloat16(64.0)
    n = np.arange(NPROJ)
    b2r = np.zeros((2, NPROJ), dtype=ml_dtypes.bfloat16)
    b2r[0] = ((n >> 6).astype(np.float32) * np.float32(2.0 ** -11)).astype(ml_dtypes.bfloat16)
    b2r[1] = ((n & 63).astype(np.float32) * np.float32(2.0 ** -17)).astype(ml_dtypes.bfloat16)

    def gtab(emb, nrow):
        t = np.zeros((128, nrow), dtype=np.float32)
        for grp in range(8):
            t[16 * grp:16 * grp + EMB] = emb.T
        return t

    def widx_layout(ids_core):
        # chunk g of 1024 ids -> partitions 16g..16g+15, wrapped i -> (i%16, i//16)
        out = np.zeros((128, 64), dtype=np.int16)
        for grp in range(8):
            ch = ids_core[grp * 1024:(grp + 1) * 1024].astype(np.int16)
            out[16 * grp:16 * grp + 16] = ch.reshape(64, 16).T
        return out

    shared = {
        "wtab": gtab(worker_emb, NW), "ptab": gtab(project_emb, NPTAB),
        "w1t": W1.T[:, :].astype(np.float32).copy(),  # [20, 40]
        "b1e": b1.reshape(HID, 1).astype(np.float32),
        "tstk": tstk, "b1r": b1r, "b2r": b2r,
        "ones3": np.ones((3, BC), dtype=ml_dtypes.bfloat16),
    }
    in_maps = []
    for core in range(NCORES):
        sl = slice(core * BC, (core + 1) * BC)
        m = dict(shared)
        m["widx"] = widx_layout(worker_ids[sl])
        m["pidx"] = widx_layout(project_ids[sl])
        in_maps.append(m)

    # host MLP for rescue rescoring
    we = worker_emb[worker_ids]
    pe = project_emb[project_ids]
    x = np.concatenate([we, pe], axis=1)
    h = np.maximum(x @ W1.T + b1, 0.0).astype(np.float32)
    w = (h @ W2.T + b2).astype(np.float32)
    return in_maps, w, table


def _decode(results, w_host, table):
    # results: list of 8 dicts with "out" [128, 512]
    v = np.zeros((B, 8), dtype=np.float32)
    for core in range(NCORES):
        o = results[core]["out"]          # [128, 8*NTILES]
        for t in range(NTILES):
            rows = slice(core * BC + t * 128, core * BC + (t + 1) * 128)
            v[rows] = o[:, 8 * t:8 * t + 8]
    k = np.rint((v.astype(np.float64) - 64.0) / EPS).astype(np.int64)
    cand = np.clip(k % 4096, 0, NPROJ - 1)
    sc = np.einsum('bkd,bd->bk', table[cand], w_host, optimize=True)
    best = cand[np.arange(B), sc.argmax(axis=1)] + 1
    return best.astype(np.int32).reshape(B, 1)


def kernel(**inputs):
    from concourse.bass_utils import run_bass_kernel_spmd
    in_maps, w_host, table = _host_prep(inputs)
    if "nc1" not in _cache:
        _cache["nc1"] = _build(L=1)
    res = run_bass_kernel_spmd(_cache["nc1"], in_maps, core_ids=list(range(NCORES)))
    return _decode(res.results, w_host, table)


# revision 7
# speedup vs baseline: 13.8952x; 13.8952x over previous
"""TRN2 Bass kernel for nn_Actor (retrieval_knn).

Data-parallel over batch across 8 NeuronCores (8192 rows/core).
Per core: ap_gather embedding lookup (feature-major), fp32 MLP layer-1 on
TensorE, then scores vs the 2489-entry table with W2 absorbed into the
table side (scores = h @ (table@W2).T + table@b2) computed as three
bf16-split pairs (h1*G1 + h1*G2 + h2*G1 + c1 + c2) for fp32-grade
precision at bf16 streaming speed; per 128-row tile a DVE max8 +
max_index gives the argmax directly. The scores phase runs under a
hardware For_i loop to keep the stored program small.
"""
import sys
sys.path.insert(0, '/opt/trn_rl_repo')
import numpy as np
import ml_dtypes

B = 65536
NCORES = 8
BC = B // NCORES            # 8192
NW, NPTAB, EMB = 1807, 2490, 10
NPROJ = NPTAB - 1           # 2489
HID = 40
NTILES = BC // 128          # 64
UNROLL = 1

_cache = {}


def _bf16(x):
    return np.asarray(x, np.float32).astype(ml_dtypes.bfloat16)


def _build(L=1):
    from concourse import bacc, mybir, bass
    from concourse.tile import TileContext
    import concourse.mybir as mb
    dt = mybir.dt
    nc = bacc.Bacc("TRN2", target_bir_lowering=False, debug=False, num_devices=NCORES)

    widx = nc.dram_tensor("widx", [128, 64], dt.int16, kind="ExternalInput")
    pidx = nc.dram_tensor("pidx", [128, 64], dt.int16, kind="ExternalInput")
    wtab16 = nc.dram_tensor("wtab16", [16, NW], dt.float32, kind="ExternalInput")
    ptab16 = nc.dram_tensor("ptab16", [16, NPTAB], dt.float32, kind="ExternalInput")
    w1t = nc.dram_tensor("w1t", [20, HID], dt.float32, kind="ExternalInput")
    b1e = nc.dram_tensor("b1e", [HID, 1], dt.float32, kind="ExternalInput")
    tstk = nc.dram_tensor("tstk", [122, NPROJ], dt.bfloat16, kind="ExternalInput")
    out_ext = nc.dram_tensor("out", [128, NTILES * 8], dt.uint32, kind="ExternalOutput")

    NIDX = BC // 8           # 1024 ids per gather group
    with TileContext(nc) as tc:
        with tc.tile_pool(name="const", bufs=1) as cp, \
             tc.tile_pool(name="work", bufs=1) as wp, \
             tc.tile_pool(name="hm", bufs=1, space="PSUM") as hmp, \
             tc.tile_pool(name="sc", bufs=1, space="PSUM") as scp:
            t_wtab = cp.tile([128, NW], dt.float32)
            t_ptab = cp.tile([128, NPTAB], dt.float32)
            t_widx = cp.tile([128, 64], dt.int16)
            t_pidx = cp.tile([128, 64], dt.int16)
            t_w1t = cp.tile([20, HID], dt.float32)
            t_b1 = cp.tile([HID, 1], dt.float32)
            t_tstk = cp.tile([122, NPROJ], dt.bfloat16)
            nc.sync.dma_start(out=t_wtab[0:16, :], in_=wtab16.ap())
            nc.sync.dma_start(out=t_ptab[0:16, :], in_=ptab16.ap())
            nc.sync.dma_start(out=t_widx, in_=widx.ap())
            nc.sync.dma_start(out=t_pidx, in_=pidx.ap())
            nc.sync.dma_start(out=t_w1t, in_=w1t.ap())
            nc.sync.dma_start(out=t_b1, in_=b1e.ap())
            nc.sync.dma_start(out=t_tstk, in_=tstk.ap())
            # replicate gather tables into all 8 groups (doubling)
            for src, n in ((t_wtab, NW), (t_ptab, NPTAB)):
                nc.gpsimd.dma_start(out=src[16:32, :], in_=src[0:16, :])
                nc.gpsimd.dma_start(out=src[32:64, :], in_=src[0:32, :])
                nc.gpsimd.dma_start(out=src[64:128, :], in_=src[0:64, :])

            wg = wp.tile([128, NIDX], dt.float32)
            pg = wp.tile([128, NIDX], dt.float32)
            x = wp.tile([20, BC], dt.float32)
            hstack = wp.tile([122, BC], dt.bfloat16)
            h_f32 = wp.tile([HID, BC], dt.float32)
            h1f = wp.tile([HID, BC], dt.float32)
            hr = wp.tile([HID, BC], dt.float32)
            onesrow = wp.tile([2, BC], dt.bfloat16)
            outbuf = wp.tile([128, NTILES * 8], dt.uint32)
            wstage = wp.tile([122, 128 * UNROLL], dt.bfloat16)
            m8 = wp.tile([128, 8 * UNROLL], dt.float32)
            nc.vector.memset(onesrow, 1.0)
            nc.gpsimd.dma_start(out=hstack[120:122, :], in_=onesrow)

            for _ in range(L):
                nc.gpsimd.ap_gather(out_ap=wg, in_ap=t_wtab, idxs_ap=t_widx,
                                    channels=128, num_elems=NW, d=1, num_idxs=NIDX)
                nc.gpsimd.ap_gather(out_ap=pg, in_ap=t_ptab, idxs_ap=t_pidx,
                                    channels=128, num_elems=NPTAB, d=1, num_idxs=NIDX)
                for g in range(8):
                    nc.gpsimd.dma_start(out=x[0:10, g * NIDX:(g + 1) * NIDX],
                                        in_=wg[16 * g:16 * g + 10, :])
                    nc.gpsimd.dma_start(out=x[10:20, g * NIDX:(g + 1) * NIDX],
                                        in_=pg[16 * g:16 * g + 10, :])
                # MLP layer 1: h = relu(x.T @ W1.T + b1), feature-major
                with tc.For_i(0, 8, 1) as cv:
                    hm = hmp.tile([HID, 1024], dt.float32)
                    nc.tensor.matmul(hm[:, 0:512], lhsT=t_w1t,
                                     rhs=x[:, bass.ds(cv * 1024, 512)],
                                     start=True, stop=True)
                    nc.tensor.matmul(hm[:, 512:1024], lhsT=t_w1t,
                                     rhs=x[:, bass.ds(cv * 1024 + 512, 512)],
                                     start=True, stop=True)
                    nc.scalar.activation(hstack[0:HID, bass.ds(cv * 1024, 1024)], hm,
                                         mb.ActivationFunctionType.Relu, bias=t_b1)
                    nc.scalar.activation(h_f32[:, bass.ds(cv * 1024, 1024)], hm,
                                         mb.ActivationFunctionType.Relu, bias=t_b1)
                # h splits: h1 = bf16(h) (done), h2 = bf16(h - h1)
                nc.gpsimd.tensor_copy(h1f, hstack[0:HID, :])
                nc.gpsimd.tensor_sub(hr, h_f32, h1f)
                nc.gpsimd.tensor_copy(hstack[64:104, :], hr)
                nc.gpsimd.dma_start(out=hstack[40:64, :], in_=hstack[0:24, :])
                nc.gpsimd.dma_start(out=hstack[104:120, :], in_=hstack[24:40, :])

                # scores + argmax under a HW loop
                with tc.For_i(0, NTILES, 1) as iv:
                    nc.vector.tensor_copy(wstage, hstack[:, bass.ds(iv * 128, 128)])
                    ps = scp.tile([128, NPROJ], dt.float32)
                    for s0 in range(0, NPROJ, 512):
                        sw = min(512, NPROJ - s0)
                        nc.tensor.matmul(ps[:, s0:s0 + sw], lhsT=wstage,
                                         rhs=t_tstk[:, s0:s0 + sw],
                                         start=True, stop=True)
                    nc.vector.max(out=m8, in_=ps)
                    nc.vector.max_index(out=outbuf[:, bass.ds(iv * 8, 8)],
                                        in_max=m8, in_values=ps)

            nc.sync.dma_start(out=out_ext.ap(), in_=outbuf)
    nc.compile()
    return nc


def _host_prep(inputs):
    worker_ids = np.asarray(inputs["worker_ids"]).astype(np.int64)
    project_ids = np.asarray(inputs["project_ids"]).astype(np.int64)
    worker_emb = np.asarray(inputs["worker_emb"], dtype=np.float32)
    project_emb = np.asarray(inputs["project_emb"], dtype=np.float32)
    W1 = np.asarray(inputs["W1"], dtype=np.float32)
    b1 = np.asarray(inputs["b1"], dtype=np.float32)
    W2 = np.asarray(inputs["W2"], dtype=np.float32)
    b2 = np.asarray(inputs["b2"], dtype=np.float32)

    table = project_emb[1:]
    G = (table @ W2).astype(np.float32)
    c = (table @ b2).astype(np.float32)
    G1 = _bf16(G)
    G2 = _bf16(G - G1.astype(np.float32))
    c1 = _bf16(c)
    c2 = _bf16(c - c1.astype(np.float32))
    tstk = np.zeros((122, NPROJ), dtype=ml_dtypes.bfloat16)
    tstk[0:40] = G1.T
    tstk[40:64] = G2.T[0:24]
    tstk[64:104] = G1.T
    tstk[104:120] = G2.T[24:40]
    tstk[120] = c1
    tstk[121] = c2

    def gtab16(emb, nrow):
        t = np.zeros((16, nrow), dtype=np.float32)
        t[0:EMB] = emb.T
        return t

    def widx_layout(ids_core):
        # [8 groups, 64 slots, 16 parts] -> [8, 16, 64] -> [128, 64]
        return ids_core.astype(np.int16).reshape(8, 64, 16).transpose(0, 2, 1).reshape(128, 64)

    shared = {
        "wtab16": gtab16(worker_emb, NW), "ptab16": gtab16(project_emb, NPTAB),
        "w1t": W1.T.astype(np.float32).copy(),
        "b1e": b1.reshape(HID, 1).astype(np.float32),
        "tstk": tstk,
    }
    in_maps = []
    for core in range(NCORES):
        sl = slice(core * BC, (core + 1) * BC)
        m = dict(shared)
        m["widx"] = widx_layout(worker_ids[sl])
        m["pidx"] = widx_layout(project_ids[sl])
        in_maps.append(m)
    return in_maps


def _decode(results):
    idx = np.zeros((B,), dtype=np.int64)
    for core in range(NCORES):
        o = results[core]["out"]          # [128, 8*NTILES] uint32
        for t in range(NTILES):
            rows = slice(core * BC + t * 128, core * BC + (t + 1) * 128)
            idx[rows] = o[:, 8 * t]
    return (idx + 1).astype(np.int32).reshape(B, 1)


def kernel(**inputs):
    from concourse.bass_utils import run_bass_kernel_spmd
    in_maps = _host_prep(inputs)
    if "nc1" not in _cache:
        _cache["nc1"] = _build(L=1)
    res = run_bass_kernel_spmd(_cache["nc1"], in_maps, core_ids=list(range(NCORES)))
    return _decode(res.results)


# revision 8
# speedup vs baseline: 16.3720x; 1.1783x over previous
"""TRN2 Bass kernel for nn_Actor (retrieval_knn).

Data-parallel over batch across 8 NeuronCores (8192 rows/core).
Per core: ap_gather embedding lookup (feature-major), fp32 MLP layer-1 on
TensorE, then scores vs the 2489-entry table with W2 absorbed into the
table side (scores = h @ (table@W2).T + table@b2) computed as three
bf16-split pairs (h1*G1 + h1*G2 + h2*G1 + c1 + c2) for fp32-grade
precision at bf16 streaming speed; per 128-row tile a DVE max8 +
max_index gives the argmax directly. The scores phase runs under a
hardware For_i loop to keep the stored program small.
"""
import sys
sys.path.insert(0, '/opt/trn_rl_repo')
import numpy as np
import ml_dtypes

B = 65536
NCORES = 8
BC = B // NCORES            # 8192
NW, NPTAB, EMB = 1807, 2490, 10
NPROJ = NPTAB - 1           # 2489
HID = 40
NTILES = BC // 128          # 64
UNROLL = 1

_cache = {}


def _bf16(x):
    return np.asarray(x, np.float32).astype(ml_dtypes.bfloat16)


def _build(L=1):
    from concourse import bacc, mybir, bass
    from concourse.tile import TileContext
    import concourse.mybir as mb
    dt = mybir.dt
    nc = bacc.Bacc("TRN2", target_bir_lowering=False, debug=False, num_devices=NCORES)

    widx = nc.dram_tensor("widx", [128, 64], dt.int16, kind="ExternalInput")
    pidx = nc.dram_tensor("pidx", [128, 64], dt.int16, kind="ExternalInput")
    wtab16 = nc.dram_tensor("wtab16", [16, NW], dt.float32, kind="ExternalInput")
    ptab16 = nc.dram_tensor("ptab16", [16, NPTAB], dt.float32, kind="ExternalInput")
    w1t = nc.dram_tensor("w1t", [20, HID], dt.float32, kind="ExternalInput")
    b1e = nc.dram_tensor("b1e", [HID, 1], dt.float32, kind="ExternalInput")
    tstk = nc.dram_tensor("tstk", [122, NPROJ], dt.bfloat16, kind="ExternalInput")
    out_ext = nc.dram_tensor("out", [128, NTILES * 8], dt.uint32, kind="ExternalOutput")

    NIDX = BC // 8           # 1024 ids per gather group
    with TileContext(nc) as tc:
        with tc.tile_pool(name="const", bufs=1) as cp, \
             tc.tile_pool(name="work", bufs=1) as wp, \
             tc.tile_pool(name="sc", bufs=1, space="PSUM") as scp:
            t_wtab = cp.tile([128, NW], dt.float32)
            t_ptab = cp.tile([128, NPTAB], dt.float32)
            t_widx = cp.tile([128, 64], dt.int16)
            t_pidx = cp.tile([128, 64], dt.int16)
            t_w1t = cp.tile([20, HID], dt.float32)
            t_b1 = cp.tile([HID, 1], dt.float32)
            t_tstk = cp.tile([122, NPROJ], dt.bfloat16)
            nc.sync.dma_start(out=t_wtab[0:16, :], in_=wtab16.ap())
            nc.sync.dma_start(out=t_ptab[0:16, :], in_=ptab16.ap())
            nc.sync.dma_start(out=t_widx, in_=widx.ap())
            nc.sync.dma_start(out=t_pidx, in_=pidx.ap())
            nc.sync.dma_start(out=t_w1t, in_=w1t.ap())
            nc.sync.dma_start(out=t_b1, in_=b1e.ap())
            nc.sync.dma_start(out=t_tstk, in_=tstk.ap())
            # replicate gather tables into all 8 groups (doubling)
            for src, n in ((t_wtab, NW), (t_ptab, NPTAB)):
                nc.gpsimd.dma_start(out=src[16:32, :], in_=src[0:16, :])
                nc.gpsimd.dma_start(out=src[32:64, :], in_=src[0:32, :])
                nc.gpsimd.dma_start(out=src[64:128, :], in_=src[0:64, :])

            wg = wp.tile([128, NIDX], dt.float32)
            pg = wp.tile([128, NIDX], dt.float32)
            x = wp.tile([20, BC], dt.float32)
            hstack = wp.tile([122, BC], dt.bfloat16)
            h_f32 = wp.tile([HID, BC], dt.float32)
            h1f = wp.tile([HID, BC], dt.float32)
            hr = wp.tile([HID, BC], dt.float32)
            onesrow = wp.tile([2, BC], dt.bfloat16)
            outbuf = wp.tile([128, NTILES * 8], dt.uint32)
            wstage = wp.tile([122, 128 * UNROLL], dt.bfloat16)
            m8 = wp.tile([128, 8 * UNROLL], dt.float32)
            nc.vector.memset(onesrow, 1.0)
            nc.gpsimd.dma_start(out=hstack[120:122, :], in_=onesrow)
            ps = scp.tile([128, NPROJ], dt.float32)

            for _ in range(L):
                nc.gpsimd.ap_gather(out_ap=wg, in_ap=t_wtab, idxs_ap=t_widx,
                                    channels=128, num_elems=NW, d=1, num_idxs=NIDX)
                nc.gpsimd.ap_gather(out_ap=pg, in_ap=t_ptab, idxs_ap=t_pidx,
                                    channels=128, num_elems=NPTAB, d=1, num_idxs=NIDX)
                for g in range(8):
                    nc.gpsimd.dma_start(out=x[0:10, g * NIDX:(g + 1) * NIDX],
                                        in_=wg[16 * g:16 * g + 10, :])
                    nc.gpsimd.dma_start(out=x[10:20, g * NIDX:(g + 1) * NIDX],
                                        in_=pg[16 * g:16 * g + 10, :])
                # MLP layer 1 (psum carved from the scores tile, 4 big chunks)
                for ch in range(4):
                    c0 = ch * 2048
                    hm = ps[0:HID, 0:2048]
                    for k in range(4):
                        nc.tensor.matmul(hm[:, k * 512:(k + 1) * 512], lhsT=t_w1t,
                                         rhs=x[:, c0 + k * 512:c0 + (k + 1) * 512],
                                         start=True, stop=True)
                    nc.scalar.activation(h_f32[:, c0:c0 + 2048], hm,
                                         mb.ActivationFunctionType.Relu, bias=t_b1)
                nc.vector.tensor_copy(hstack[0:HID, :], h_f32)
                # h splits: h1 = bf16(h) (done), h2 = bf16(h - h1)
                nc.gpsimd.tensor_copy(h1f, hstack[0:HID, :])
                nc.gpsimd.tensor_sub(hr, h_f32, h1f)
                nc.gpsimd.tensor_copy(hstack[64:104, :], hr)
                nc.gpsimd.dma_start(out=hstack[40:64, :], in_=hstack[0:24, :])
                nc.gpsimd.dma_start(out=hstack[104:120, :], in_=hstack[24:40, :])

                # scores + argmax under a HW loop
                with tc.For_i(0, NTILES, 1) as iv:
                    nc.vector.tensor_copy(wstage, hstack[:, bass.ds(iv * 128, 128)])
                    ps = scp.tile([128, NPROJ], dt.float32)
                    for s0 in range(0, NPROJ, 512):
                        sw = min(512, NPROJ - s0)
                        nc.tensor.matmul(ps[:, s0:s0 + sw], lhsT=wstage,
                                         rhs=t_tstk[:, s0:s0 + sw],
                                         start=True, stop=True)
                    nc.vector.max(out=m8, in_=ps)
                    nc.vector.max_index(out=outbuf[:, bass.ds(iv * 8, 8)],
                                        in_max=m8, in_values=ps)

            nc.sync.dma_start(out=out_ext.ap(), in_=outbuf)
    nc.compile()
    return nc


def _host_prep(inputs):
    worker_ids = np.asarray(inputs["worker_ids"]).astype(np.int64)
    project_ids = np.asarray(inputs["project_ids"]).astype(np.int64)
    worker_emb = np.asarray(inputs["worker_emb"], dtype=np.float32)
    project_emb = np.asarray(inputs["project_emb"], dtype=np.float32)
    W1 = np.asarray(inputs["W1"], dtype=np.float32)
    b1 = np.asarray(inputs["b1"], dtype=np.float32)
    W2 = np.asarray(inputs["W2"], dtype=np.float32)
    b2 = np.asarray(inputs["b2"], dtype=np.float32)

    table = project_emb[1:]
    G = (table @ W2).astype(np.float32)
    c = (table @ b2).astype(np.float32)
    G1 = _bf16(G)
    G2 = _bf16(G - G1.astype(np.float32))
    c1 = _bf16(c)
    c2 = _bf16(c - c1.astype(np.float32))
    tstk = np.zeros((122, NPROJ), dtype=ml_dtypes.bfloat16)
    tstk[0:40] = G1.T
    tstk[40:64] = G2.T[0:24]
    tstk[64:104] = G1.T
    tstk[104:120] = G2.T[24:40]
    tstk[120] = c1
    tstk[121] = c2

    def gtab16(emb, nrow):
        t = np.zeros((16, nrow), dtype=np.float32)
        t[0:EMB] = emb.T
        return t

    def widx_layout(ids_core):
        # [8 groups, 64 slots, 16 parts] -> [8, 16, 64] -> [128, 64]
        return ids_core.astype(np.int16).reshape(8, 64, 16).transpose(0, 2, 1).reshape(128, 64)

    shared = {
        "wtab16": gtab16(worker_emb, NW), "ptab16": gtab16(project_emb, NPTAB),
        "w1t": W1.T.astype(np.float32).copy(),
        "b1e": b1.reshape(HID, 1).astype(np.float32),
        "tstk": tstk,
    }
    in_maps = []
    for core in range(NCORES):
        sl = slice(core * BC, (core + 1) * BC)
        m = dict(shared)
        m["widx"] = widx_layout(worker_ids[sl])
        m["pidx"] = widx_layout(project_ids[sl])
        in_maps.append(m)
    return in_maps


def _decode(results):
    idx = np.zeros((B,), dtype=np.int64)
    for core in range(NCORES):
        o = results[core]["out"]          # [128, 8*NTILES] uint32
        for t in range(NTILES):
            rows = slice(core * BC + t * 128, core * BC + (t + 1) * 128)
            idx[rows] = o[:, 8 * t]
    return (idx + 1).astype(np.int32).reshape(B, 1)


def kernel(**inputs):
    from concourse.bass_utils import run_bass_kernel_spmd
    in_maps = _host_prep(inputs)
    if "nc1" not in _cache:
        _cache["nc1"] = _build(L=1)
    res = run_bass_kernel_spmd(_cache["nc1"], in_maps, core_ids=list(range(NCORES)))
    return _decode(res.results)
